# revision 15
# baseline (speedup 1.0000x reference)
"""Distributed attention kernel for Trainium2 (8 NeuronCores).

Sharding: B*H = 2*16 = 32 (batch, head) pairs over 8 cores.
Core c handles batch b = c//4 and global heads 4*(c%4) .. 4*(c%4)+3
(i.e. output columns (c%4)*256 : (c%4+1)*256 of the 1024-wide output).

Per-core kernel (compute in bf16, f32 PSUM accumulation):
  - inputs arrive pre-transposed from host: qT/kT/vT [1024, 2048] bf16,
    weight column slices wq/wk/wv [1024, 256] bf16, mask columns msk
    [128, 16] f32 (0/1), identity [128, 128] f32, sel [128, 2] f32
    (sumexp gather matrix: rows {0,64}->col0, {32,96}->col1).
  - projections: QWT/KWT in [d, s] layout, VW in [s, d] layout with
    mask-zeroed rows for masked keys (implements the additive key mask).
  - scores computed transposed: S_T[k, q]; the two heads of a projection
    tile are row-packed (64x128 PE tiling) so head pairs stream
    concurrently through disjoint PE row groups.
  - exp split between ScalarE (table exp, bf16 out) and VectorE
    (Schraudolph bit-trick exp: bf16 bits = round(x*A + B) written as
    int16; the constant-factor approximation error cancels in softmax).
  - PV col-tiled: per key-chunk the two heads' [128,64] V tiles occupy
    PE column halves (tile_position (0,0)/(0,64)) and run concurrently,
    accumulating O_T [128=2*64 d, 512 q] per chunk in one PSUM bank.
  - sumexp via M=1 matmuls with the mask column as stationary, 4-up
    col-tiled at positions (0,0/32/64/96) over (kc parity, head).
  - epilogue: copy O_T to SBUF bf16, PE-transpose 128x128 subtiles,
    fold the parity-pair sumexp merge into a PE transpose against `sel`,
    reciprocal + per-partition scale on VectorE, DMA out (t-major
    layout, host reassembles).
"""

import numpy as np

HEADS = 16
DK = 64
DM = 1024
B = 2
S = 2048
HL = 4           # heads per core
NCOL = HL * DK   # 256 projection cols per core
NM = DM // 128   # 8 m-chunks
NKC = S // 128   # 16 k-chunks
NQC = S // 512   # 4 q-chunks
NSUB = 512 // 128
NBLK = 2 * NKC   # 32 score blocks of 512 per (t, qc); 2 blocks per s-tile
NST = NBLK // 2  # 16 s-tiles per (t, qc)

# Schraudolph exp in bf16 bit space: bits = x*SCH_A + SCH_B (then int16
# round); includes the 1/sqrt(dk)=0.125 score scale.
SCH_A = 1.4426950408889634 * 128.0 * 0.125
SCH_B = 127.0 * 128.0 - 7.0

_CACHE = {}
# which kc tiles go to VectorE exp, per chunk index 0..7
DVE_KCS = {
    0: (),
    1: (5, 11),
    2: (1, 3, 5, 8, 10, 12, 14),
    3: (1, 3, 5, 8, 10, 12, 14),
    4: (1, 3, 5, 8, 10, 12, 14),
    5: (1, 3, 5, 8, 10, 12, 14),
    6: (1, 3, 5, 8, 10, 12, 14),
    7: (1, 3, 5, 8, 10, 12, 14),
}


def _build():
    from contextlib import ExitStack
    import concourse.bass as bass  # noqa: F401
    import concourse.mybir as mybir
    import concourse.bacc as bacc
    import concourse.tile as tile
    from concourse.alu_op_type import AluOpType

    f32 = mybir.dt.float32
    bf16 = mybir.dt.bfloat16
    i16 = mybir.dt.int16
    Exp = mybir.ActivationFunctionType.Exp

    nc = bacc.Bacc("TRN2", target_bir_lowering=False, debug=False, num_devices=8)

    qT = nc.dram_tensor("qT", [DM, S], bf16, kind="ExternalInput").ap()
    kT = nc.dram_tensor("kT", [DM, S], bf16, kind="ExternalInput").ap()
    vT = nc.dram_tensor("vT", [DM, S], bf16, kind="ExternalInput").ap()
    # weights arrive host-swizzled to the SBUF layout [128, NM*NCOL]
    wq = nc.dram_tensor("wq", [128, NM * NCOL], bf16, kind="ExternalInput").ap()
    wk = nc.dram_tensor("wk", [128, NM * NCOL], bf16, kind="ExternalInput").ap()
    wv = nc.dram_tensor("wv", [128, NM * NCOL], bf16, kind="ExternalInput").ap()
    msk = nc.dram_tensor("msk", [128, NKC], f32, kind="ExternalInput").ap()
    ident = nc.dram_tensor("ident", [128, 128], f32, kind="ExternalInput").ap()
    sel = nc.dram_tensor("sel", [128, 2], f32, kind="ExternalInput").ap()
    # t-major output: rows [t*2048 + q], 128 cols (heads 2t, 2t+1)
    out = nc.dram_tensor("out", [2 * S, 128], f32, kind="ExternalOutput").ap()

    with tile.TileContext(nc) as tc, ExitStack() as ctx:
        const = ctx.enter_context(tc.tile_pool(name="const", bufs=1))
        bigp = ctx.enter_context(tc.tile_pool(name="bigp", bufs=3, space="PSUM"))
        # ov and t2 share one bank: the transposes only touch it after
        # the O_T copy has drained the PV result
        ovt2 = ctx.enter_context(tc.tile_pool(name="ovt2", bufs=1, space="PSUM"))
        sep = ctx.enter_context(tc.tile_pool(name="sep", bufs=1, space="PSUM"))
        ep = ctx.enter_context(tc.tile_pool(name="ep", bufs=30))
        otsp = ctx.enter_context(tc.tile_pool(name="otsp", bufs=2))
        serowp = ctx.enter_context(tc.tile_pool(name="serowp", bufs=2))
        rcp = ctx.enter_context(tc.tile_pool(name="rcp", bufs=4))
        outp = ctx.enter_context(tc.tile_pool(name="outp", bufs=8))

        # ---- persistent SBUF tensors ----
        xq = const.tile([128, NM * S], bf16, tag="xq")
        xk = const.tile([128, NM * S], bf16, tag="xk")
        xv = const.tile([128, NM * S], bf16, tag="xv")
        wq_sb = const.tile([128, NM * NCOL], bf16, tag="wq")
        wk_sb = const.tile([128, NM * NCOL], bf16, tag="wk")
        wv_sb = const.tile([128, NM * NCOL], bf16, tag="wv")
        m_sb = const.tile([128, NKC], f32, tag="m")
        mskb = const.tile([128, NKC], bf16, tag="mb")
        id_sb = const.tile([128, 128], f32, tag="id")
        id_b = const.tile([128, 128], bf16, tag="idb")
        sel_sb = const.tile([128, 2], f32, tag="sel")
        sel_b = const.tile([128, 2], bf16, tag="selb")
        qwt = const.tile([128, 2 * S], bf16, tag="qwt")    # [d(2 heads), s] x2
        kwt = const.tile([128, 2 * S], bf16, tag="kwt")
        vw = const.tile([128, NKC * HL * DK], bf16, tag="vw")
        vw_4d = vw[:, :].rearrange("p (k h c) -> p k h c", k=NKC, h=HL)

        # persistent PSUM sumexp accumulator (rows 0/32/64/96)
        ses = sep.tile([128, 512], f32, tag="ses", name="ses")

        # ---- input DMA: weights/consts, then xk/xq interleaved, then xv ----
        # wk arrives in m-chunks so the first projection matmul only waits
        # for wk-m0 + xk-m0 instead of the full wk tensor
        nc.sync.dma_start(out=wk_sb[:, 0:NCOL], in_=wk[:, 0:NCOL])
        nc.sync.dma_start(out=xk[:, 0:S], in_=kT[0:128, :])
        for m in range(1, NM):
            nc.sync.dma_start(
                out=wk_sb[:, m * NCOL: (m + 1) * NCOL],
                in_=wk[:, m * NCOL: (m + 1) * NCOL],
            )
            nc.sync.dma_start(
                out=xk[:, m * S: (m + 1) * S], in_=kT[m * 128: (m + 1) * 128, :]
            )
        for m in range(NM):
            nc.sync.dma_start(
                out=wq_sb[:, m * NCOL: (m + 1) * NCOL],
                in_=wq[:, m * NCOL: (m + 1) * NCOL],
            )
            nc.sync.dma_start(
                out=xq[:, m * S: (m + 1) * S], in_=qT[m * 128: (m + 1) * 128, :]
            )
        for m in range(NM):
            nc.sync.dma_start(
                out=xv[:, m * S: (m + 1) * S], in_=vT[m * 128: (m + 1) * 128, :]
            )
        nc.sync.dma_start(out=wv_sb[:, :], in_=wv)
        nc.sync.dma_start(out=m_sb[:, :], in_=msk)
        nc.sync.dma_start(out=id_sb[:, :], in_=ident)
        nc.sync.dma_start(out=sel_sb[:, :], in_=sel)

        # one-time derived constants + sumexp-bank scrub (rows between the
        # 4 accumulator rows are read by the epilogue copy and must not be
        # NaN; matmul start=True only clears rows it writes)
        nc.vector.memset(ses[:, :], 0.0)
        nc.vector.tensor_copy(mskb[:, :], m_sb[:, :])
        nc.vector.tensor_copy(id_b[:, :], id_sb[:, :])
        nc.vector.tensor_copy(sel_b[:, :], sel_sb[:, :])

        def proj_qk(w_sb, x_sb, dst, t, qc, pool=None, ptag="big"):
            pool = pool or bigp
            ps = pool.tile([128, 512], f32, tag=ptag, name=f"pqk{t}_{qc}")
            for m in range(NM):
                nc.tensor.matmul(
                    ps[:, :],
                    lhsT=w_sb[:, m * NCOL + t * 128: m * NCOL + t * 128 + 128],
                    rhs=x_sb[:, m * S + qc * 512: m * S + qc * 512 + 512],
                    start=(m == 0),
                    stop=(m == NM - 1),
                )
            nc.vector.tensor_copy(
                dst[:, t * S + qc * 512: t * S + qc * 512 + 512], ps[:, :]
            )

        def proj_qk_mouter(w_sb, x_sb, dst, t):
            # m-outer: consume each x chunk as its DMA lands (4 live tiles)
            ps = [
                bigp.tile([128, 512], f32, tag="big", name=f"pm{t}_{q}")
                for q in (0, 1, 2)
            ] + [
                ovt2.tile([128, 512], f32, tag="ov", name=f"pm{t}_3")
            ]
            for m in range(NM):
                for qc in range(NQC):
                    nc.tensor.matmul(
                        ps[qc][:, :],
                        lhsT=w_sb[:, m * NCOL + t * 128: m * NCOL + t * 128 + 128],
                        rhs=x_sb[:, m * S + qc * 512: m * S + qc * 512 + 512],
                        start=(m == 0),
                        stop=(m == NM - 1),
                    )
                    if m == NM - 1:
                        nc.vector.tensor_copy(
                            dst[:, t * S + qc * 512: t * S + qc * 512 + 512],
                            ps[qc][:, :],
                        )

        def proj_v(kb):
            ps = bigp.tile([128, NCOL], f32, tag="big", name=f"pv{kb}")
            for m in range(NM):
                nc.tensor.matmul(
                    ps[:, :],
                    lhsT=xv[:, m * S + kb * 128: m * S + kb * 128 + 128],
                    rhs=wv_sb[:, m * NCOL: (m + 1) * NCOL],
                    start=(m == 0),
                    stop=(m == NM - 1),
                )
            nc.vector.tensor_scalar_mul(
                vw_4d[:, kb, :, :],
                ps[:, :].rearrange("p (h c) -> p h c", h=HL),
                m_sb[:, kb: kb + 1],
            )

        class Chunk:
            """Incremental emitter for one (t, qc) attention chunk."""

            def __init__(self, ci, t, qc):
                self.ci, self.t, self.qc = ci, t, qc
                self.s_tiles = [None] * NST
                self.e_tiles = [None] * NST
                self.ov = None
                self.si = 0
                self.pi = 0
                self.zi = 0

            def emit_s(self, n):
                t, qc = self.t, self.qc
                todo = list(range(self.si, min(self.si + n, NBLK)))
                if not todo:
                    return
                self.si = todo[-1] + 1
                for blk in todo:
                    if blk % 2 == 0:
                        self.s_tiles[blk // 2] = bigp.tile(
                            [128, 1024], f32, tag="big", name=f"sps{t}_{qc}_{blk}"
                        )
                for blk in todo:
                    kc, a = divmod(blk, 2)
                    nc.tensor.matmul(
                        self.s_tiles[kc][:, a * 512: (a + 1) * 512],
                        lhsT=kwt[
                            64 * a: 64 * a + 64,
                            t * S + kc * 128: t * S + kc * 128 + 128,
                        ],
                        rhs=qwt[
                            64 * a: 64 * a + 64,
                            t * S + qc * 512: t * S + qc * 512 + 512,
                        ],
                        start=True,
                        stop=True,
                        tile_position=(64 * a, 0),
                    )
                for blk in todo:
                    kc, a = divmod(blk, 2)
                    if a != 1:
                        continue
                    et = ep.tile(
                        [128, 1024], bf16, tag="e", name=f"et{t}_{qc}_{kc}"
                    )
                    self.e_tiles[kc] = et
                    if kc in DVE_KCS[self.ci]:
                        nc.vector.tensor_scalar(
                            out=et[:, :].bitcast(i16),
                            in0=self.s_tiles[kc][:, :],
                            scalar1=SCH_A,
                            scalar2=SCH_B,
                            op0=AluOpType.mult,
                            op1=AluOpType.add,
                        )
                    else:
                        nc.scalar.activation(
                            et[:, :], self.s_tiles[kc][:, :], Exp, scale=0.125
                        )

            def emit_pv(self, n):
                t = self.t
                if self.ov is None:
                    self.ov = ovt2.tile(
                        [128, 512], f32, tag="ov", name=f"ov{t}_{self.qc}"
                    )
                blks = list(range(self.pi, min(self.pi + n, NBLK)))
                if not blks:
                    return
                self.pi = blks[-1] + 1
                for blk in blks:
                    kc, a = divmod(blk, 2)
                    # two interleaved per-head accumulation chains share this
                    # bank on disjoint partition halves; per-element
                    # has_written handles it, but the sim's coarse group
                    # checker must be bypassed
                    nc.tensor.matmul(
                        self.ov[64 * a: 64 * a + 64, :],
                        lhsT=vw_4d[:, kc, 2 * t + a, :],
                        rhs=self.e_tiles[kc][:, a * 512: (a + 1) * 512],
                        start=(kc == 0),
                        stop=(kc == NKC - 1),
                        skip_group_check=True,
                        tile_position=(0, 64 * a),
                    )

            def emit_se(self, n):
                # sumexp slot p covers kcs (2p, 2p+1) x 2 heads, 4-up
                # col-tiled at positions 0/32/64/96
                slots = list(range(self.zi, min(self.zi + n, NKC // 2)))
                if not slots:
                    return
                self.zi = slots[-1] + 1
                for p in slots:
                    for j, (kc, a) in enumerate(
                        ((2 * p, 0), (2 * p, 1), (2 * p + 1, 0), (2 * p + 1, 1))
                    ):
                        nc.tensor.matmul(
                            ses[32 * j: 32 * j + 1, :],
                            lhsT=mskb[:, kc: kc + 1],
                            rhs=self.e_tiles[kc][:, a * 512: (a + 1) * 512],
                            start=(p == 0),
                            stop=(p == NKC // 2 - 1),
                            tile_position=(0, 32 * j),
                        )

        def epilogue(ch):
            t, qc = ch.t, ch.qc
            # O_T and sumexp to SBUF (PE cannot read PSUM)
            ovs = otsp.tile([128, 512], bf16, tag="ots", name=f"ovs{t}_{qc}")
            nc.vector.tensor_copy(ovs[:, :], ch.ov[:, :])
            serow = serowp.tile([97, 512], bf16, tag="ser", name=f"ser{t}_{qc}")
            nc.vector.tensor_copy(serow[:, :], ses[0:97, :])
            # transpose O_T subtiles and gather/merge sumexp via sel
            t2 = ovt2.tile([128, 1024], bf16, tag="ov", name=f"t2{t}_{qc}")
            t2_3d = t2[:, 0:512].rearrange("p (s c) -> p s c", s=NSUB)
            t2se = t2[:, 512:528].bitcast(f32).rearrange(
                "p (s c) -> p s c", s=NSUB
            )
            for sub in range(NSUB):
                # all 8 writes share the t2 bank as independent start/stop
                # singles on disjoint ranges -> bypass the coarse group check
                nc.tensor.matmul(
                    t2_3d[:, sub, :],
                    lhsT=ovs[:, sub * 128: (sub + 1) * 128],
                    rhs=id_b[:, :],
                    is_transpose=True,
                    skip_group_check=True,
                )
                # sumexp gather+merge: [97,128]^T @ sel -> [128 q, 2 heads];
                # sel sums the kc-parity accumulator pair per head
                nc.tensor.matmul(
                    t2se[:, sub, :],
                    lhsT=serow[:, sub * 128: (sub + 1) * 128],
                    rhs=sel_b[0:97, :],
                    start=True,
                    stop=True,
                    skip_group_check=True,
                )
            rc = rcp.tile([128, NSUB * 2], f32, tag="rc", name=f"rc{t}_{qc}")
            nc.vector.reciprocal_approx_fast(
                out=rc[:, :].rearrange("p (s c) -> p s c", s=NSUB),
                in_=t2se[:, :, :],
            )
            for sub in range(NSUB):
                o_out = outp.tile([128, 128], f32, tag="out", name=f"oo{t}_{qc}_{sub}")
                for a in range(2):
                    # ScalarE has slack here; keeping these off VectorE stops
                    # them from delaying the DVE exp tiles that free score
                    # buffers (PE stalls otherwise)
                    nc.scalar.mul(
                        o_out[:, a * 64: (a + 1) * 64],
                        t2_3d[:, sub, a * 64: (a + 1) * 64],
                        rc[:, 2 * sub + a: 2 * sub + a + 1],
                    )
                nc.sync.dma_start(
                    out=out[
                        t * S + qc * 512 + sub * 128:
                        t * S + qc * 512 + sub * 128 + 128, :
                    ],
                    in_=o_out[:, :],
                )

        # ---- schedule ----
        # warmup: K/Q t0 projections woven with chunk0 scores; then rounds
        # of [scores x4, pv x4, se x1] so ScalarE/VectorE exp never starves
        # and PE mode switches amortize over 4-matmul runs.
        chunks = [Chunk(ci, ci // 4, ci % 4) for ci in range(8)]

        proj_qk_mouter(wk_sb, xk, kwt, 0)
        for qc in range(NQC):
            proj_qk(wk_sb, xk, kwt, 1, qc)   # runs while xq still arriving
        proj_qk_mouter(wq_sb, xq, qwt, 0)
        for r in range(8):
            chunks[0].emit_s(4)
            if r < 4:
                # Q t1 projection fills chunk-0's exp-paced PE idle
                proj_qk(wq_sb, xq, qwt, 1, r)
            elif r >= 5:
                proj_v(r - 5)   # first V tiles as xv lands
        # V projection + chunk1 scores + chunk0 PV
        for i in range(8):
            if i < 6:
                proj_v(2 * i + 3)
                proj_v(2 * i + 4)
            elif i == 6:
                proj_v(15)
            chunks[0].emit_pv(4)
            if i >= 1:
                chunks[0].emit_se(1)
            chunks[1].emit_s(4)
        chunks[0].emit_se(NKC // 2)
        epilogue(chunks[0])
        # t1 projections + chunk2 scores + chunk1 PV
        for j in range(8):
            chunks[2].emit_s(4)
            if j >= 1:
                chunks[1].emit_pv(4)
            if j >= 2:
                chunks[1].emit_se(1)
        chunks[1].emit_pv(4)
        chunks[1].emit_se(NKC // 2)
        epilogue(chunks[1])
        # steady state: rounds of [next-chunk scores x4, current PV x4,
        # current se x1]; the final chunk's PV weaves into the
        # second-to-last chunk's rounds
        for ci in range(2, 7):
            for i in range(8):
                chunks[ci + 1].emit_s(4)
                if i >= 1:
                    chunks[ci].emit_se(1)
                if i >= 2:
                    chunks[ci].emit_pv(4)
                    if ci == 6:
                        chunks[7].emit_pv(4)
            chunks[ci].emit_pv(8)
            chunks[ci].emit_se(NKC // 2)
            epilogue(chunks[ci])
        chunks[7].emit_pv(NBLK)
        chunks[7].emit_se(NKC // 2)
        epilogue(chunks[7])

    nc.compile()
    return nc


def _get_nc():
    if "nc" not in _CACHE:
        _CACHE["nc"] = _build()
    return _CACHE["nc"]


def _shard_inputs(q, k, v, mask, Wq, Wk, Wv):
    import ml_dtypes

    bf16 = ml_dtypes.bfloat16
    q = np.asarray(q, np.float32)
    k = np.asarray(k, np.float32)
    v = np.asarray(v, np.float32)
    mask = np.asarray(mask, np.float32)
    Wq = np.asarray(Wq, np.float32)
    Wk = np.asarray(Wk, np.float32)
    Wv = np.asarray(Wv, np.float32)

    def _swz(w):
        # [1024, 256] -> SBUF layout [128, 8*256] (row p = concat_m W[m*128+p])
        return np.ascontiguousarray(
            w.reshape(NM, 128, NCOL).transpose(1, 0, 2).reshape(128, NM * NCOL)
        ).astype(bf16)

    ident = np.eye(128, dtype=np.float32)
    sel = np.zeros((128, 2), np.float32)
    sel[0, 0] = sel[64, 0] = 1.0
    sel[32, 1] = sel[96, 1] = 1.0
    qTs = [np.ascontiguousarray(q[b].T).astype(bf16) for b in range(B)]
    kTs = [np.ascontiguousarray(k[b].T).astype(bf16) for b in range(B)]
    vTs = [np.ascontiguousarray(v[b].T).astype(bf16) for b in range(B)]
    msks = [
        np.ascontiguousarray(mask[b].reshape(NKC, 128).T).astype(np.float32)
        for b in range(B)
    ]
    in_maps = []
    for c in range(8):
        b, j = c // 4, c % 4
        sl = slice(j * NCOL, (j + 1) * NCOL)
        in_maps.append(
            {
                "qT": qTs[b],
                "kT": kTs[b],
                "vT": vTs[b],
                "wq": _swz(Wq[:, sl]),
                "wk": _swz(Wk[:, sl]),
                "wv": _swz(Wv[:, sl]),
                "msk": msks[b],
                "ident": ident,
                "sel": sel,
            }
        )
    return in_maps


def _assemble(results):
    """results: list of 8 dicts with 'out' [2*S, 128] -> full [B, S, 1024]."""
    outp = np.empty((B, S, HEADS * DK), np.float32)
    for c in range(8):
        b, j = c // 4, c % 4
        o = np.asarray(results[c]["out"]).reshape(2, S, 128)
        outp[b, :, j * NCOL: j * NCOL + 128] = o[0]
        outp[b, :, j * NCOL + 128: j * NCOL + 256] = o[1]
    return outp


def kernel(q, k, v, mask, Wq, Wk, Wv):
    from concourse.bass_utils import run_bass_kernel_spmd

    nc = _get_nc()
    in_maps = _shard_inputs(q, k, v, mask, Wq, Wk, Wv)
    res = run_bass_kernel_spmd(nc, in_maps, core_ids=list(range(8))).results
    return _assemble(res)


# revision 16
# speedup vs baseline: 1.1085x; 1.1085x over previous
"""Distributed attention kernel for Trainium2 (8 NeuronCores).

Sharding: B*H = 2*16 = 32 (batch, head) pairs over 8 cores.
Core c handles batch b = c//4 and global heads 4*(c%4) .. 4*(c%4)+3
(i.e. output columns (c%4)*256 : (c%4+1)*256 of the 1024-wide output).

Per-core kernel (compute in bf16, f32 PSUM accumulation):
  - inputs arrive pre-transposed from host: qT/kT/vT [1024, 2048] bf16,
    weight column slices wq/wk/wv [1024, 256] bf16, mask columns msk
    [128, 16] f32 (0/1), identity [128, 128] f32, sel [128, 2] f32
    (sumexp gather matrix: rows {0,64}->col0, {32,96}->col1).
  - projections: QWT/KWT in [d, s] layout, VW in [s, d] layout with
    mask-zeroed rows for masked keys (implements the additive key mask).
  - scores computed transposed: S_T[k, q]; the two heads of a projection
    tile are row-packed (64x128 PE tiling) so head pairs stream
    concurrently through disjoint PE row groups.
  - exp split between ScalarE (table exp, bf16 out) and VectorE
    (Schraudolph bit-trick exp: bf16 bits = round(x*A + B) written as
    int16; the constant-factor approximation error cancels in softmax).
  - PV col-tiled: per key-chunk the two heads' [128,64] V tiles occupy
    PE column halves (tile_position (0,0)/(0,64)) and run concurrently,
    accumulating O_T [128=2*64 d, 512 q] per chunk in one PSUM bank.
  - sumexp via M=1 matmuls with the mask column as stationary, 4-up
    col-tiled at positions (0,0/32/64/96) over (kc parity, head).
  - epilogue: copy O_T to SBUF bf16, PE-transpose 128x128 subtiles,
    fold the parity-pair sumexp merge into a PE transpose against `sel`,
    reciprocal + per-partition scale on VectorE, DMA out (t-major
    layout, host reassembles).
"""

import numpy as np

HEADS = 16
DK = 64
DM = 1024
B = 2
S = 2048
HL = 4           # heads per core
NCOL = HL * DK   # 256 projection cols per core
NM = DM // 128   # 8 m-chunks
NKC = S // 128   # 16 k-chunks
NQC = S // 512   # 4 q-chunks
NSUB = 512 // 128
NBLK = 2 * NKC   # 32 score blocks of 512 per (t, qc); 2 blocks per s-tile
NST = NBLK // 2  # 16 s-tiles per (t, qc)

# Schraudolph exp in bf16 bit space: bits = x*SCH_A + SCH_B (then int16
# round); includes the 1/sqrt(dk)=0.125 score scale.
SCH_A = 1.4426950408889634 * 128.0 * 0.125
SCH_B = 127.0 * 128.0 - 7.0

_CACHE = {}
# which kc tiles go to VectorE exp, per chunk index 0..7
DVE_KCS = {
    0: (),
    1: (5, 11),
    2: (1, 3, 5, 8, 10, 12, 14),
    3: (1, 3, 5, 8, 10, 12, 14),
    4: (1, 3, 5, 8, 10, 12, 14),
    5: (1, 3, 5, 8, 10, 12, 14),
    6: (1, 3, 5, 8, 10, 12, 14),
    7: (1, 3, 5, 8, 10, 12, 14),
}


def _build():
    from contextlib import ExitStack
    import concourse.bass as bass  # noqa: F401
    import concourse.mybir as mybir
    import concourse.bacc as bacc
    import concourse.tile as tile
    from concourse.alu_op_type import AluOpType

    f32 = mybir.dt.float32
    bf16 = mybir.dt.bfloat16
    i16 = mybir.dt.int16
    Exp = mybir.ActivationFunctionType.Exp

    nc = bacc.Bacc("TRN2", target_bir_lowering=False, debug=False, num_devices=8)

    qT = nc.dram_tensor("qT", [DM, S], bf16, kind="ExternalInput").ap()
    kT = nc.dram_tensor("kT", [DM, S], bf16, kind="ExternalInput").ap()
    vT = nc.dram_tensor("vT", [DM, S], bf16, kind="ExternalInput").ap()
    # weights arrive host-swizzled to the SBUF layout [128, NM*NCOL]
    wq = nc.dram_tensor("wq", [128, NM * NCOL], bf16, kind="ExternalInput").ap()
    wk = nc.dram_tensor("wk", [128, NM * NCOL], bf16, kind="ExternalInput").ap()
    wv = nc.dram_tensor("wv", [128, NM * NCOL], bf16, kind="ExternalInput").ap()
    msk = nc.dram_tensor("msk", [128, NKC], f32, kind="ExternalInput").ap()
    ident = nc.dram_tensor("ident", [128, 128], f32, kind="ExternalInput").ap()
    sel = nc.dram_tensor("sel", [128, 2], f32, kind="ExternalInput").ap()
    # t-major output: rows [t*2048 + q], 128 cols (heads 2t, 2t+1)
    out = nc.dram_tensor("out", [2 * S, 128], f32, kind="ExternalOutput").ap()

    with tile.TileContext(nc) as tc, ExitStack() as ctx:
        const = ctx.enter_context(tc.tile_pool(name="const", bufs=1))
        bigp = ctx.enter_context(tc.tile_pool(name="bigp", bufs=5, space="PSUM"))
        # ov and t2 share one bank: the transposes only touch it after
        # the O_T copy has drained the PV result
        ovt2 = ctx.enter_context(tc.tile_pool(name="ovt2", bufs=2, space="PSUM"))
        sep = ctx.enter_context(tc.tile_pool(name="sep", bufs=1, space="PSUM"))
        ep = ctx.enter_context(tc.tile_pool(name="ep", bufs=56))
        otsp = ctx.enter_context(tc.tile_pool(name="otsp", bufs=2))
        serowp = ctx.enter_context(tc.tile_pool(name="serowp", bufs=2))
        rcp = ctx.enter_context(tc.tile_pool(name="rcp", bufs=4))
        outp = ctx.enter_context(tc.tile_pool(name="outp", bufs=8))

        # ---- persistent SBUF tensors ----
        xq = const.tile([128, NM * S], bf16, tag="xq")
        xk = const.tile([128, NM * S], bf16, tag="xk")
        xv = const.tile([128, NM * S], bf16, tag="xv")
        wq_sb = const.tile([128, NM * NCOL], bf16, tag="wq")
        wk_sb = const.tile([128, NM * NCOL], bf16, tag="wk")
        wv_sb = const.tile([128, NM * NCOL], bf16, tag="wv")
        m_sb = const.tile([128, NKC], f32, tag="m")
        mskb = const.tile([128, NKC], bf16, tag="mb")
        id_sb = const.tile([128, 128], f32, tag="id")
        id_b = const.tile([128, 128], bf16, tag="idb")
        sel_sb = const.tile([128, 2], f32, tag="sel")
        sel_b = const.tile([128, 2], bf16, tag="selb")
        qwt = const.tile([128, 2 * S], bf16, tag="qwt")    # [d(2 heads), s] x2
        kwt = const.tile([128, 2 * S], bf16, tag="kwt")
        vw = const.tile([128, NKC * HL * DK], bf16, tag="vw")
        vw_4d = vw[:, :].rearrange("p (k h c) -> p k h c", k=NKC, h=HL)

        # persistent PSUM sumexp accumulator (rows 0/32/64/96)
        ses = sep.tile([128, 512], f32, tag="ses", name="ses")

        # ---- input DMA: weights/consts, then xk/xq interleaved, then xv ----
        # wk arrives in m-chunks so the first projection matmul only waits
        # for wk-m0 + xk-m0 instead of the full wk tensor
        nc.sync.dma_start(out=wk_sb[:, 0:NCOL], in_=wk[:, 0:NCOL])
        nc.sync.dma_start(out=xk[:, 0:S], in_=kT[0:128, :])
        for m in range(1, NM):
            nc.sync.dma_start(
                out=wk_sb[:, m * NCOL: (m + 1) * NCOL],
                in_=wk[:, m * NCOL: (m + 1) * NCOL],
            )
            nc.sync.dma_start(
                out=xk[:, m * S: (m + 1) * S], in_=kT[m * 128: (m + 1) * 128, :]
            )
        for m in range(NM):
            nc.sync.dma_start(
                out=wq_sb[:, m * NCOL: (m + 1) * NCOL],
                in_=wq[:, m * NCOL: (m + 1) * NCOL],
            )
            nc.sync.dma_start(
                out=xq[:, m * S: (m + 1) * S], in_=qT[m * 128: (m + 1) * 128, :]
            )
        for m in range(NM):
            nc.sync.dma_start(
                out=xv[:, m * S: (m + 1) * S], in_=vT[m * 128: (m + 1) * 128, :]
            )
        nc.sync.dma_start(out=wv_sb[:, :], in_=wv)
        nc.sync.dma_start(out=m_sb[:, :], in_=msk)
        nc.sync.dma_start(out=id_sb[:, :], in_=ident)
        nc.sync.dma_start(out=sel_sb[:, :], in_=sel)

        # one-time derived constants + sumexp-bank scrub (rows between the
        # 4 accumulator rows are read by the epilogue copy and must not be
        # NaN; matmul start=True only clears rows it writes)
        nc.vector.memset(ses[:, :], 0.0)
        nc.vector.tensor_copy(mskb[:, :], m_sb[:, :])
        nc.vector.tensor_copy(id_b[:, :], id_sb[:, :])
        nc.vector.tensor_copy(sel_b[:, :], sel_sb[:, :])

        def proj_qk(w_sb, x_sb, dst, t, qc, pool=None, ptag="big"):
            pool = pool or bigp
            ps = pool.tile([128, 512], f32, tag=ptag, name=f"pqk{t}_{qc}")
            for m in range(NM):
                nc.tensor.matmul(
                    ps[:, :],
                    lhsT=w_sb[:, m * NCOL + t * 128: m * NCOL + t * 128 + 128],
                    rhs=x_sb[:, m * S + qc * 512: m * S + qc * 512 + 512],
                    start=(m == 0),
                    stop=(m == NM - 1),
                )
            nc.vector.tensor_copy(
                dst[:, t * S + qc * 512: t * S + qc * 512 + 512], ps[:, :]
            )

        def proj_qk_mouter(w_sb, x_sb, dst, t):
            # m-outer: consume each x chunk as its DMA lands (4 live tiles)
            ps = [
                bigp.tile([128, 512], f32, tag="big", name=f"pm{t}_{q}")
                for q in range(NQC)
            ]
            for m in range(NM):
                for qc in range(NQC):
                    nc.tensor.matmul(
                        ps[qc][:, :],
                        lhsT=w_sb[:, m * NCOL + t * 128: m * NCOL + t * 128 + 128],
                        rhs=x_sb[:, m * S + qc * 512: m * S + qc * 512 + 512],
                        start=(m == 0),
                        stop=(m == NM - 1),
                    )
                    if m == NM - 1:
                        nc.vector.tensor_copy(
                            dst[:, t * S + qc * 512: t * S + qc * 512 + 512],
                            ps[qc][:, :],
                        )

        def proj_v(kb):
            ps = bigp.tile([128, NCOL], f32, tag="big", name=f"pv{kb}")
            for m in range(NM):
                nc.tensor.matmul(
                    ps[:, :],
                    lhsT=xv[:, m * S + kb * 128: m * S + kb * 128 + 128],
                    rhs=wv_sb[:, m * NCOL: (m + 1) * NCOL],
                    start=(m == 0),
                    stop=(m == NM - 1),
                )
            nc.vector.tensor_scalar_mul(
                vw_4d[:, kb, :, :],
                ps[:, :].rearrange("p (h c) -> p h c", h=HL),
                m_sb[:, kb: kb + 1],
            )

        class Chunk:
            """Incremental emitter for one (t, qc) attention chunk."""

            def __init__(self, ci, t, qc):
                self.ci, self.t, self.qc = ci, t, qc
                self.s_tiles = [None] * NBLK
                self.e_tiles = [None] * NBLK
                self.ov = None
                self.si = 0
                self.pi = 0
                self.zi = 0

            def emit_s(self, n):
                t, qc = self.t, self.qc
                todo = list(range(self.si, min(self.si + n, NBLK)))
                if not todo:
                    return
                self.si = todo[-1] + 1
                for blk in todo:
                    self.s_tiles[blk] = bigp.tile(
                        [128, 512], f32, tag="big", name=f"sps{t}_{qc}_{blk}"
                    )
                for blk in todo:
                    kc, a = divmod(blk, 2)
                    nc.tensor.matmul(
                        self.s_tiles[blk][:, :],
                        lhsT=kwt[
                            64 * a: 64 * a + 64,
                            t * S + kc * 128: t * S + kc * 128 + 128,
                        ],
                        rhs=qwt[
                            64 * a: 64 * a + 64,
                            t * S + qc * 512: t * S + qc * 512 + 512,
                        ],
                        start=True,
                        stop=True,
                        tile_position=(64 * a, 0),
                    )
                for blk in todo:
                    kc, a = divmod(blk, 2)
                    et = ep.tile(
                        [128, 512], bf16, tag="e", name=f"et{t}_{qc}_{blk}"
                    )
                    self.e_tiles[blk] = et
                    if kc in DVE_KCS[self.ci]:
                        nc.vector.tensor_scalar(
                            out=et[:, :].bitcast(i16),
                            in0=self.s_tiles[blk][:, :],
                            scalar1=SCH_A,
                            scalar2=SCH_B,
                            op0=AluOpType.mult,
                            op1=AluOpType.add,
                        )
                    else:
                        nc.scalar.activation(
                            et[:, :], self.s_tiles[blk][:, :], Exp, scale=0.125
                        )

            def emit_pv(self, n):
                t = self.t
                if self.ov is None:
                    self.ov = ovt2.tile(
                        [128, 512], f32, tag="ov", name=f"ov{t}_{self.qc}"
                    )
                blks = list(range(self.pi, min(self.pi + n, NBLK)))
                if not blks:
                    return
                self.pi = blks[-1] + 1
                for blk in blks:
                    kc, a = divmod(blk, 2)
                    # two interleaved per-head accumulation chains share this
                    # bank on disjoint partition halves; per-element
                    # has_written handles it, but the sim's coarse group
                    # checker must be bypassed
                    nc.tensor.matmul(
                        self.ov[64 * a: 64 * a + 64, :],
                        lhsT=vw_4d[:, kc, 2 * t + a, :],
                        rhs=self.e_tiles[blk][:, :],
                        start=(kc == 0),
                        stop=(kc == NKC - 1),
                        skip_group_check=True,
                        tile_position=(0, 64 * a),
                    )

            def emit_se(self, n):
                # sumexp slot p covers kcs (2p, 2p+1) x 2 heads, 4-up
                # col-tiled at positions 0/32/64/96
                slots = list(range(self.zi, min(self.zi + n, NKC // 2)))
                if not slots:
                    return
                self.zi = slots[-1] + 1
                for p in slots:
                    for j, (kc, a) in enumerate(
                        ((2 * p, 0), (2 * p, 1), (2 * p + 1, 0), (2 * p + 1, 1))
                    ):
                        nc.tensor.matmul(
                            ses[32 * j: 32 * j + 1, :],
                            lhsT=mskb[:, kc: kc + 1],
                            rhs=self.e_tiles[2 * kc + a][:, :],
                            start=(p == 0),
                            stop=(p == NKC // 2 - 1),
                            tile_position=(0, 32 * j),
                        )

        def epilogue(ch):
            t, qc = ch.t, ch.qc
            # O_T and sumexp to SBUF (PE cannot read PSUM)
            ovs = otsp.tile([128, 512], bf16, tag="ots", name=f"ovs{t}_{qc}")
            nc.vector.tensor_copy(ovs[:, :], ch.ov[:, :])
            serow = serowp.tile([97, 512], bf16, tag="ser", name=f"ser{t}_{qc}")
            nc.vector.tensor_copy(serow[:, :], ses[0:97, :])
            # transpose O_T subtiles and gather/merge sumexp via sel
            t2 = ovt2.tile([128, 1024], bf16, tag="ov", name=f"t2{t}_{qc}")
            t2_3d = t2[:, 0:512].rearrange("p (s c) -> p s c", s=NSUB)
            t2se = t2[:, 512:528].bitcast(f32).rearrange(
                "p (s c) -> p s c", s=NSUB
            )
            for sub in range(NSUB):
                # all 8 writes share the t2 bank as independent start/stop
                # singles on disjoint ranges -> bypass the coarse group check
                nc.tensor.matmul(
                    t2_3d[:, sub, :],
                    lhsT=ovs[:, sub * 128: (sub + 1) * 128],
                    rhs=id_b[:, :],
                    is_transpose=True,
                    skip_group_check=True,
                )
                # sumexp gather+merge: [97,128]^T @ sel -> [128 q, 2 heads];
                # sel sums the kc-parity accumulator pair per head
                nc.tensor.matmul(
                    t2se[:, sub, :],
                    lhsT=serow[:, sub * 128: (sub + 1) * 128],
                    rhs=sel_b[0:97, :],
                    start=True,
                    stop=True,
                    skip_group_check=True,
                )
            rc = rcp.tile([128, NSUB * 2], f32, tag="rc", name=f"rc{t}_{qc}")
            nc.vector.reciprocal_approx_fast(
                out=rc[:, :].rearrange("p (s c) -> p s c", s=NSUB),
                in_=t2se[:, :, :],
            )
            for sub in range(NSUB):
                o_out = outp.tile([128, 128], f32, tag="out", name=f"oo{t}_{qc}_{sub}")
                for a in range(2):
                    nc.vector.tensor_scalar_mul(
                        o_out[:, a * 64: (a + 1) * 64],
                        t2_3d[:, sub, a * 64: (a + 1) * 64],
                        rc[:, 2 * sub + a: 2 * sub + a + 1],
                    )
                nc.sync.dma_start(
                    out=out[
                        t * S + qc * 512 + sub * 128:
                        t * S + qc * 512 + sub * 128 + 128, :
                    ],
                    in_=o_out[:, :],
                )

        # ---- schedule ----
        # warmup: K/Q t0 projections woven with chunk0 scores; then rounds
        # of [scores x4, pv x4, se x1] so ScalarE/VectorE exp never starves
        # and PE mode switches amortize over 4-matmul runs.
        chunks = [Chunk(ci, ci // 4, ci % 4) for ci in range(8)]

        proj_qk_mouter(wk_sb, xk, kwt, 0)
        for qc in range(NQC):
            proj_qk(wk_sb, xk, kwt, 1, qc)   # runs while xq still arriving
        proj_qk_mouter(wq_sb, xq, qwt, 0)
        for r in range(8):
            chunks[0].emit_s(4)
            if r < 4:
                # Q t1 projection fills chunk-0's exp-paced PE idle
                proj_qk(wq_sb, xq, qwt, 1, r)
            elif r >= 5:
                proj_v(r - 5)   # first V tiles as xv lands
        # V projection + chunk1 scores + chunk0 PV
        for i in range(8):
            if i < 6:
                proj_v(2 * i + 3)
                proj_v(2 * i + 4)
            elif i == 6:
                proj_v(15)
            chunks[0].emit_pv(4)
            if i >= 1:
                chunks[0].emit_se(1)
            chunks[1].emit_s(4)
        chunks[0].emit_se(NKC // 2)
        epilogue(chunks[0])
        # t1 projections + chunk2 scores + chunk1 PV
        for j in range(8):
            chunks[2].emit_s(4)
            if j >= 1:
                chunks[1].emit_pv(4)
            if j >= 2:
                chunks[1].emit_se(1)
        chunks[1].emit_pv(4)
        chunks[1].emit_se(NKC // 2)
        epilogue(chunks[1])
        # steady state: rounds of [next-chunk scores x4, current PV x4,
        # current se x1]; the final chunk's PV weaves into the
        # second-to-last chunk's rounds
        for ci in range(2, 7):
            for i in range(8):
                chunks[ci + 1].emit_s(4)
                if i >= 2:
                    chunks[ci].emit_pv(4)
                    if ci == 6:
                        chunks[7].emit_pv(4)
                if i >= 3:
                    chunks[ci].emit_se(1)
            chunks[ci].emit_pv(8)
            chunks[ci].emit_se(NKC // 2)
            epilogue(chunks[ci])
        chunks[7].emit_pv(NBLK)
        chunks[7].emit_se(NKC // 2)
        epilogue(chunks[7])

    nc.compile()
    return nc


def _get_nc():
    if "nc" not in _CACHE:
        _CACHE["nc"] = _build()
    return _CACHE["nc"]


def _shard_inputs(q, k, v, mask, Wq, Wk, Wv):
    import ml_dtypes

    bf16 = ml_dtypes.bfloat16
    q = np.asarray(q, np.float32)
    k = np.asarray(k, np.float32)
    v = np.asarray(v, np.float32)
    mask = np.asarray(mask, np.float32)
    Wq = np.asarray(Wq, np.float32)
    Wk = np.asarray(Wk, np.float32)
    Wv = np.asarray(Wv, np.float32)

    def _swz(w):
        # [1024, 256] -> SBUF layout [128, 8*256] (row p = concat_m W[m*128+p])
        return np.ascontiguousarray(
            w.reshape(NM, 128, NCOL).transpose(1, 0, 2).reshape(128, NM * NCOL)
        ).astype(bf16)

    ident = np.eye(128, dtype=np.float32)
    sel = np.zeros((128, 2), np.float32)
    sel[0, 0] = sel[64, 0] = 1.0
    sel[32, 1] = sel[96, 1] = 1.0
    qTs = [np.ascontiguousarray(q[b].T).astype(bf16) for b in range(B)]
    kTs = [np.ascontiguousarray(k[b].T).astype(bf16) for b in range(B)]
    vTs = [np.ascontiguousarray(v[b].T).astype(bf16) for b in range(B)]
    msks = [
        np.ascontiguousarray(mask[b].reshape(NKC, 128).T).astype(np.float32)
        for b in range(B)
    ]
    in_maps = []
    for c in range(8):
        b, j = c // 4, c % 4
        sl = slice(j * NCOL, (j + 1) * NCOL)
        in_maps.append(
            {
                "qT": qTs[b],
                "kT": kTs[b],
                "vT": vTs[b],
                "wq": _swz(Wq[:, sl]),
                "wk": _swz(Wk[:, sl]),
                "wv": _swz(Wv[:, sl]),
                "msk": msks[b],
                "ident": ident,
                "sel": sel,
            }
        )
    return in_maps


def _assemble(results):
    """results: list of 8 dicts with 'out' [2*S, 128] -> full [B, S, 1024]."""
    outp = np.empty((B, S, HEADS * DK), np.float32)
    for c in range(8):
        b, j = c // 4, c % 4
        o = np.asarray(results[c]["out"]).reshape(2, S, 128)
        outp[b, :, j * NCOL: j * NCOL + 128] = o[0]
        outp[b, :, j * NCOL + 128: j * NCOL + 256] = o[1]
    return outp


def kernel(q, k, v, mask, Wq, Wk, Wv):
    from concourse.bass_utils import run_bass_kernel_spmd

    nc = _get_nc()
    in_maps = _shard_inputs(q, k, v, mask, Wq, Wk, Wv)
    res = run_bass_kernel_spmd(nc, in_maps, core_ids=list(range(8))).results
    return _assemble(res)
